# revision 10
# baseline (speedup 1.0000x reference)
"""Trainium2 Bass kernel: DeformableValueAttention (exp-spine schedule, v2).

Full-input contract: kernel(**inputs) takes the unsharded inputs of
reference.setup_inputs() and returns the full [B, C, H, W] output.

Sharding: 8 cores = (batch b, head-group g). Each core computes 4 of the 8
attention heads for ALL 1024 queries of one batch and produces a PARTIAL
[C, N] output (its 4 heads' contribution through Wo, fp16); the host sums
the two partials per batch in fp32.

v2 schedule ("exp spine"): the ACT engine's 32 Exp tiles (~1.07us each on
[128,1024] fp32 PSUM -> bf16) are the hard serial resource (~34us); the PE
work (~36us at 2.4GHz) is arranged so the spine never stalls:

  A (t~2.5-5us)  QT(hp0,qf0) + KT(hp0) chunk0 as soon as their DMA lands.
  B (spine hp0)  16 units: S-pair(hp0,m,qf) -> exp. PE backfill between
                 pairs: KT(hp0) JIT chunks, QT(hp0,qf1), QT/KT(hp1), V, Vd.
  C (spine hp1)  qf-outer: 8 units (qf0,m0..7), then 8 (qf1,m0..7).
                 PE backfill: O(hp0) (all m, qf-grouped) -> norm(hp0) ->
                 O(hp1) lag-behind; norm(hp1,qf0) closes mid-C.
  D (tail ~5us)  last O pair -> Wo(qf0) || norm(hp1,qf1) -> Wo(qf1),
                 per-pt pipelined with fp16 copies + out DMA.

PSUM (8 banks): tag "ps_s" 2 x [128,1024] (4 banks) for scores; tag
"ps_acc" 4 x [<=128,<=512] (4 banks) time-shared: projections/V/Vd (B) ->
O accumulators of one head-pair at a time (C) -> Wo chunks (D).

Engines: PE matmuls; ACT only Exp (one table load); DVE reciprocal (read
straight from the accumulator's ones-row in PSUM), norm multiply, output
fp16 copies, Vd copies; Pool partition-broadcasts, QT/KT/V copies, and the
xkv/gt DMA queue. Input DMAs are priority-ordered and chunked so the first
S pair fires ~4us in.

Notes on fidelity vs reference.py:
  - P_thermal adds a per-query constant to scores pre-softmax; softmax is
    exactly invariant to that, so it is skipped.
  - All biases in setup_inputs() are zeros; nonzero biases or off-spec
    shapes fall back to a numpy reference implementation.
"""

import sys

import numpy as np
import ml_dtypes

try:
    import concourse.bass as bass  # noqa: F401
except ImportError:  # pragma: no cover - path fallback for bare containers
    sys.path.insert(0, "/opt/trn_rl_repo")
    import concourse.bass as bass  # noqa: F401

import concourse.bacc as bacc
import concourse.tile as tile
from concourse import mybir
from concourse.bass_utils import run_bass_kernel_spmd

B, C, HH, WW = 4, 512, 32, 32
N = HH * WW          # 1024 spatial positions = keys = queries
NH, HD = 8, 64       # total heads, head dim
G = 2                # head groups (cores per batch)
HG = NH // G         # heads per core (4)
CG = HG * HD         # channels per core (256)
P = 128
CT = C // P          # 4 input-channel partition-tiles
NKT = N // P         # 8 key tiles
NCORES = 8
BF16 = mybir.dt.bfloat16
FP16 = mybir.dt.float16
FP32 = mybir.dt.float32
NP_BF16 = ml_dtypes.bfloat16
EXP = mybir.ActivationFunctionType.Exp


# --------------------------------------------------------------------------
# host-side helpers
# --------------------------------------------------------------------------

def _gather_T(offsets_b, salf_b):
    """GT[k, n]: weight of source pixel k in grid-sampled output pixel n,
    with the per-source value modulation salf folded in. fp32 [N, N]."""
    ys = np.linspace(-1.0, 1.0, HH)
    xs = np.linspace(-1.0, 1.0, WW)
    gy, gx = np.meshgrid(ys, xs, indexing="ij")
    x = ((gx + offsets_b[0] / (WW / 2.0) + 1.0) * WW - 1.0) * 0.5
    y = ((gy + offsets_b[1] / (HH / 2.0) + 1.0) * HH - 1.0) * 0.5
    x = np.clip(x, 0.0, WW - 1.0)
    y = np.clip(y, 0.0, HH - 1.0)
    x0 = np.floor(x); y0 = np.floor(y)
    wx = x - x0; wy = y - y0
    x0i = x0.astype(np.int64); y0i = y0.astype(np.int64)
    x1i = np.minimum(x0i + 1, WW - 1); y1i = np.minimum(y0i + 1, HH - 1)
    GT = np.zeros((N, N), np.float32)
    n_idx = np.arange(N)
    for yi, xi, w in ((y0i, x0i, (1 - wx) * (1 - wy)),
                      (y0i, x1i, wx * (1 - wy)),
                      (y1i, x0i, (1 - wx) * wy),
                      (y1i, x1i, wx * wy)):
        np.add.at(GT, ((yi * WW + xi).reshape(-1), n_idx),
                  w.reshape(-1).astype(np.float32))
    GT *= salf_b[:, None]
    return GT


def _reference_numpy(q_feat, kv_feat, offsets, saliency_map, P_thermal,
                     Wq, bq, Wk, bk, Wv, bv, Wo, bo, lambda_p, gamma_val):
    """Plain numpy port of reference.py -- correctness fallback only."""
    Bq, Cq = q_feat.shape[0], q_feat.shape[1]
    Nq = q_feat.shape[2] * q_feat.shape[3]
    qf = q_feat.reshape(Bq, Cq, Nq).transpose(0, 2, 1)
    kf = kv_feat.reshape(Bq, Cq, Nq).transpose(0, 2, 1)

    def heads(x, Wm, bm):
        return (x @ Wm.T + bm).reshape(Bq, Nq, NH, -1).transpose(0, 2, 1, 3)

    Q = heads(qf, Wq, bq)
    K = heads(kf, Wk, bk)
    V = heads(kf, Wv, bv)
    hd = Cq // NH
    attn = np.einsum("bhqd,bhkd->bhqk", Q, K) * (hd ** -0.5)
    attn = attn + float(lambda_p) * P_thermal.reshape(Bq, 1, Nq, 1)
    attn = attn - attn.max(axis=-1, keepdims=True)
    w = np.exp(attn)
    w /= w.sum(axis=-1, keepdims=True)
    Vm = V * (1.0 + float(gamma_val) * saliency_map.reshape(Bq, 1, Nq, 1))
    Vsp = Vm.transpose(0, 2, 1, 3).reshape(Bq, Nq, Cq).transpose(0, 2, 1)
    Vd = np.empty_like(Vsp)
    for b in range(Bq):
        GT = _gather_T(offsets[b], np.ones(Nq, np.float32))
        Vd[b] = Vsp[b] @ GT
    Vdf = Vd.reshape(Bq, Cq, Nq).transpose(0, 2, 1).reshape(Bq, Nq, NH, hd).transpose(0, 2, 1, 3)
    out = np.einsum("bhqk,bhkd->bhqd", w, Vdf)
    out = out.transpose(0, 2, 1, 3).reshape(Bq, Nq, Cq)
    out = out @ Wo.T + bo
    return out.transpose(0, 2, 1).reshape(q_feat.shape).astype(np.float32)


# --------------------------------------------------------------------------
# device program
# --------------------------------------------------------------------------

def _build_program(chunks):
    """chunks: ordered list of (m, k) gather-tile pairs; same for all cores."""
    nch = len(chunks)
    chunks_for_m = {m: [] for m in range(NKT)}
    for idx, (m, k) in enumerate(chunks):
        chunks_for_m[m].append((idx, k))

    nc = bacc.Bacc(None, target_bir_lowering=False, debug=False)
    xq_d = nc.declare_dram_parameter("xq", [P, CT * N], BF16, isOutput=False)
    xkv_d = nc.declare_dram_parameter("xkv", [P, CT * N], BF16,
                                      isOutput=False)
    wq_d = nc.declare_dram_parameter("wqT", [P, CT * CG], BF16,
                                     isOutput=False)
    wk_d = nc.declare_dram_parameter("wkT", [P, CT * CG], BF16,
                                     isOutput=False)
    wv_d = nc.declare_dram_parameter("wvT", [P, CT * CG], BF16,
                                     isOutput=False)
    wo_d = nc.declare_dram_parameter("woT", [P, G * C], BF16, isOutput=False)
    gt_d = nc.declare_dram_parameter("gt", [P, nch * P], BF16,
                                     isOutput=False)
    out_d = nc.declare_dram_parameter("outT", [C, N], FP16, isOutput=True)

    with tile.TileContext(nc) as tc:
        with tc.tile_pool(name="const", bufs=1) as const, \
             tc.tile_pool(name="work", bufs=1) as work, \
             tc.tile_pool(name="pu_pool", bufs=1) as pu_pool, \
             tc.tile_pool(name="sm", bufs=4) as sm, \
             tc.tile_pool(name="psp", bufs=2, space="PSUM") as psp:

            # ---- SBUF input tiles (host pre-tiled to [128, X] layouts) ----
            xq_sb = const.tile([P, CT * N], BF16, name="xq", tag="xq")
            xkv_sb = const.tile([P, CT * N], BF16, name="xkv", tag="xkv")
            wq_sb = const.tile([P, CT * CG], BF16, name="wq", tag="wq")
            wk_sb = const.tile([P, CT * CG], BF16, name="wk", tag="wk")
            wv_sb = const.tile([P, CT * CG], BF16, name="wv", tag="wv")
            gt_w = const.tile([P, nch * P], BF16, name="gtw", tag="gtw")
            wo_sb = const.tile([P, G * C], BF16, name="wo", tag="wo")

            # ---- input DMAs, priority-ordered per queue -------------------
            # sync: xq qf0 then qf1 (QT(hp0,qf0) starts ~3us in); scalar: the
            # weight stack (ACT is idle until the first exp); pool: xkv in
            # 256-col chunks (KT m-chunks go just-in-time), then gt.
            def colchunk(t, lo, hi):
                return t[:].rearrange("p (k n) -> p k n", n=N)[:, :, lo:hi]

            nc.sync.dma_start(out=colchunk(xq_sb, 0, 512),
                              in_=colchunk(xq_d, 0, 512))
            nc.sync.dma_start(out=colchunk(xq_sb, 512, 1024),
                              in_=colchunk(xq_d, 512, 1024))
            for w_d, w_sb in ((wq_d, wq_sb), (wk_d, wk_sb), (wv_d, wv_sb)):
                nc.scalar.dma_start(out=w_sb[:], in_=w_d[:])
            nc.gpsimd.dma_start(out=colchunk(xkv_sb, 0, 256),
                                in_=colchunk(xkv_d, 0, 256))

            # ---- SBUF result tiles ----------------------------------------
            qt_sb = {hp: work.tile([P, N], BF16, name=f"qt{hp}", tag=f"qt{hp}")
                     for hp in range(G)}
            kt_sb = {hp: work.tile([P, N], BF16, name=f"kt{hp}", tag=f"kt{hp}")
                     for hp in range(G)}
            v_sb = {}
            vd_sb = {}
            o_sb = {hp: work.tile([P, N], BF16, name=f"o{hp}", tag=f"o{hp}")
                    for hp in range(G)}
            pu_tiles = {}
            ps_o = {}

            # ---- emission helpers -----------------------------------------
            def emit_qt_chunk(hp, qf):
                # qt_sb[hp][:, qf*512:] = (Wq_hp @ xq)[:, qf half]
                cols = slice(qf * 512, (qf + 1) * 512)
                ps = psp.tile([P, 512], FP32, name=f"psq{hp}{qf}",
                              tag="ps_acc", bufs=4)
                for k in range(CT):
                    nc.tensor.matmul(
                        ps[:],
                        lhsT=wq_sb[:, k * CG + hp * P:k * CG + (hp + 1) * P],
                        rhs=xq_sb[:, k * N + qf * 512:k * N + (qf + 1) * 512],
                        start=(k == 0), stop=(k == CT - 1))
                nc.vector.tensor_copy(qt_sb[hp][:, cols], ps[:])

            def emit_kt_chunk(hp, j):
                # kt_sb[hp][:, j*256:(j+1)*256] (key-tile pair 2j, 2j+1)
                cols = slice(j * 256, (j + 1) * 256)
                ps = psp.tile([P, 256], FP32, name=f"psk{hp}{j}",
                              tag="ps_acc", bufs=4)
                for k in range(CT):
                    nc.tensor.matmul(
                        ps[:],
                        lhsT=wk_sb[:, k * CG + hp * P:k * CG + (hp + 1) * P],
                        rhs=xkv_sb[:, k * N + j * 256:k * N + (j + 1) * 256],
                        start=(k == 0), stop=(k == CT - 1))
                nc.vector.tensor_copy(kt_sb[hp][:, cols], ps[:])

            def emit_v(m):
                ps = psp.tile([P, CG], FP32, name=f"psv{m}", tag="ps_acc",
                              bufs=4)
                for k in range(CT):
                    nc.tensor.matmul(ps[:],
                                     lhsT=xkv_sb[:, k * N + m * P:k * N + (m + 1) * P],
                                     rhs=wv_sb[:, k * CG:(k + 1) * CG],
                                     start=(k == 0), stop=(k == CT - 1))
                tl = work.tile([P, CG], BF16, name=f"v{m}", tag=f"v{m}")
                nc.vector.tensor_copy(tl[:], ps[:])
                v_sb[m] = tl

            def emit_vd(m):
                ps = psp.tile([P, CG], FP32, name=f"psvd{m}", tag="ps_acc",
                              bufs=4)
                lst = chunks_for_m[m]
                for j, (idx, k) in enumerate(lst):
                    nc.tensor.matmul(ps[:],
                                     lhsT=gt_w[:, idx * P:(idx + 1) * P],
                                     rhs=v_sb[k][:],
                                     start=(j == 0), stop=(j == len(lst) - 1))
                tl = work.tile([P, HG * (HD + 1)], BF16,
                               name=f"vd{m}", tag=f"vd{m}")
                tl3 = tl[:].rearrange("p (h e) -> p h e", e=HD + 1)
                nc.vector.tensor_copy(
                    tl3[:, :, 0:HD],
                    ps[:].rearrange("p (h e) -> p h e", e=HD))
                nc.vector.memset(tl3[:, :, HD:HD + 1], 1.0)
                vd_sb[m] = tl

            def emit_s(hp, m, qf):
                # scores for both heads of pair hp, key-tile m, query half
                # qf; exp straight off PSUM into a bf16 pu tile.
                kt, qt = kt_sb[hp], qt_sb[hp]
                ps_s = psp.tile([P, N], FP32, name=f"pss{hp}{m}{qf}",
                                tag="ps_s", bufs=2)
                nc.tensor.matmul(
                    ps_s[:, 0:512],
                    lhsT=kt[0:HD, m * P:(m + 1) * P],
                    rhs=qt[0:HD, qf * 512:(qf + 1) * 512],
                    start=True, stop=True)
                nc.tensor.matmul(
                    ps_s[:, 512:1024],
                    lhsT=kt[HD:P, m * P:(m + 1) * P],
                    rhs=qt[HD:P, qf * 512:(qf + 1) * 512],
                    start=True, stop=True)
                pu = pu_pool.tile([P, N], BF16, name=f"pu{hp}{m}{qf}",
                                  tag="pu", bufs=22)
                nc.scalar.activation(out=pu[:], in_=ps_s[:], func=EXP)
                pu_tiles[(hp, m, qf)] = pu

            def alloc_ps_o(hp, qf):
                for hh in range(2):
                    ps_o[(hp, hh, qf)] = psp.tile(
                        [HD + 1, 512], FP32, name=f"pso{hp}{hh}{qf}",
                        tag="ps_acc", bufs=4)

            def emit_o(hp, m, qf):
                vd3 = vd_sb[m][:].rearrange("p (h e) -> p h e", e=HD + 1)
                pu = pu_tiles[(hp, m, qf)]
                for hh in range(2):
                    nc.tensor.matmul(
                        ps_o[(hp, hh, qf)][:],
                        lhsT=vd3[:, 2 * hp + hh, :],
                        rhs=pu[:, hh * 512:(hh + 1) * 512],
                        start=(m == 0), stop=(m == NKT - 1))

            def emit_norm(hp, qf):
                # o_sb[hp][0:64, qf half] = head 2hp, [64:128] = head 2hp+1,
                # each row block scaled by its softmax reciprocal. The
                # reciprocal reads the accumulator's ones-row straight from
                # PSUM; Pool broadcasts it across the 64 head partitions.
                cols = slice(qf * 512, (qf + 1) * 512)
                recs = []
                for hh in range(2):
                    dn = sm.tile([1, 512], FP32, name=f"dn{hp}{hh}{qf}",
                                 tag="dn", bufs=4)
                    nc.vector.tensor_copy(dn[:],
                                          ps_o[(hp, hh, qf)][HD:HD + 1, :])
                    rec = sm.tile([1, 512], FP32, name=f"rec{hp}{hh}{qf}",
                                  tag="rec", bufs=4)
                    nc.vector.reciprocal_approx_fast(rec[:], dn[:])
                    recs.append(rec)
                for hh in range(2):
                    bc = sm.tile([HD, 512], FP32, name=f"bc{hp}{hh}{qf}",
                                 tag="bc", bufs=2)
                    nc.gpsimd.partition_broadcast(bc[:], recs[hh][:])
                    nc.vector.tensor_mul(o_sb[hp][hh * HD:(hh + 1) * HD, cols],
                                         ps_o[(hp, hh, qf)][0:HD, :],
                                         bc[:])

            def emit_wo(qf, pts=range(CT), ob_eng="scalar"):
                cols = slice(qf * 512, (qf + 1) * 512)
                for pt in pts:
                    ps = psp.tile([P, 512], FP32, name=f"psw{pt}{qf}",
                                  tag="ps_acc", bufs=4)
                    for hp in range(G):
                        nc.tensor.matmul(
                            ps[:],
                            lhsT=wo_sb[:, hp * C + pt * P:
                                       hp * C + (pt + 1) * P],
                            rhs=o_sb[hp][:, cols],
                            start=(hp == 0), stop=(hp == G - 1))
                    ob = sm.tile([P, 512], FP16, name=f"ob{pt}{qf}",
                                 tag="ob", bufs=2)
                    if ob_eng == "vector":
                        nc.vector.tensor_copy(ob[:], ps[:])
                    else:
                        nc.scalar.copy(ob[:], ps[:])
                    nc.sync.dma_start(out=out_d[pt * P:(pt + 1) * P, cols],
                                      in_=ob[:])

            # ---- emission schedule ----------------------------------------
            # Phase A: minimal prologue for the first S pair. Wave-2 DMAs
            # (xkv c1-3, gt, wo) are held on the gpsimd queue behind a tiny
            # copy that reads kt_sb, so the critical wave-1 transfers
            # (wq, xq-qf0, wk, xkv-c0) get the full DMA bandwidth first.
            emit_qt_chunk(0, 0)
            emit_kt_chunk(0, 0)
            gate = sm.tile([1, 2], BF16, name="gate", tag="gate", bufs=1)
            nc.gpsimd.tensor_copy(gate[0:1, 0:1], kt_sb[0][0:1, 0:1])
            for j in range(1, 4):
                nc.gpsimd.dma_start(
                    out=colchunk(xkv_sb, j * 256, (j + 1) * 256),
                    in_=colchunk(xkv_d, j * 256, (j + 1) * 256))
            nc.gpsimd.dma_start(out=gt_w[:], in_=gt_d[:])
            nc.gpsimd.dma_start(out=wo_sb[:], in_=wo_d[:])

            # Phase B: hp0 spine (qf-outer: all qf0 units then all qf1), PE
            # backfill ordered by DMA arrival and need-by unit: KT(hp0) JIT
            # chunks, QT(hp0,qf1) before unit 8, QT/KT(hp1), V, Vd.
            backfill = ([lambda j=j: emit_kt_chunk(0, j) for j in (1, 2, 3)]
                        + [lambda: emit_qt_chunk(0, 1),
                           lambda: emit_qt_chunk(1, 0),
                           lambda: emit_qt_chunk(1, 1)]
                        + [lambda j=j: emit_kt_chunk(1, j) for j in range(4)]
                        + [lambda m=m: emit_v(m) for m in range(NKT)]
                        + [lambda m=m: emit_vd(m) for m in range(NKT)])
            # backfill items to run after spine unit u (16 units, 26 items):
            # KT0 JIT (c1 by unit 2, c2 by 4, c3 by 6), QT0qf1 by unit 8,
            # V from unit ~5 (wv lands ~6us), Vd from unit ~9 (gt ~10us).
            steps = [1, 1, 1, 1, 2, 2, 2, 2, 2, 2, 2, 2, 2, 2, 1, 1]
            bi = 0
            for u, (qf, m) in enumerate([(qf, m) for qf in range(2)
                                         for m in range(NKT)]):
                emit_s(0, m, qf)
                for _ in range(steps[u]):
                    if bi < len(backfill):
                        backfill[bi]()
                        bi += 1
            while bi < len(backfill):
                backfill[bi]()
                bi += 1

            # Phase C: hp1 spine (qf-outer). PE backfill: O(hp0) qf-grouped
            # then norm(hp0); O(hp1) lags once its accumulators are free.
            c_spine = [(qf, m) for qf in range(2) for m in range(NKT)]
            # O(hp0): all 16 (m, qf) pairs, qf-grouped, 2 per spine unit.
            alloc_ps_o(0, 0)
            alloc_ps_o(0, 1)
            o0_fill = ([(0, m, 0) for m in range(NKT)]
                       + [(0, m, 1) for m in range(NKT)])
            o0i = 0
            o1_ready = []          # (hp1, m, qf) O units whose pu exists
            o1i = 0
            norm0_done = False
            o1_alloc = [False, False]

            def drain_o1(limit):
                nonlocal o1i
                while o1i < len(o1_ready) and o1i < limit:
                    m, qf = o1_ready[o1i]
                    if not o1_alloc[qf]:
                        alloc_ps_o(1, qf)
                        o1_alloc[qf] = True
                    emit_o(1, m, qf)
                    o1i += 1

            norm1q0_done = False
            for u, (qf, m) in enumerate(c_spine):
                emit_s(1, m, qf)
                o1_ready.append((m, qf))
                # PE backfill, paced at ~2 O units per spine unit:
                # u0-7: O(hp0) with norm(0,qf) at each qf's completion;
                # u8-15: O(hp1) lag-2 stream, norm(1,0) at qf0 completion,
                # then the first half of Wo(qf0).
                if u < 8:
                    for _ in range(2):
                        emit_o(*o0_fill[o0i])
                        o0i += 1
                    if o0i == 8:
                        emit_norm(0, 0)
                    elif o0i == 16:
                        emit_norm(0, 1)
                elif u < 12:
                    drain_o1(o1i + 2)
                    if o1i >= 8 and not norm1q0_done:
                        emit_norm(1, 0)    # qf0 accumulators complete
                        norm1q0_done = True
                elif u < 14:
                    drain_o1(o1i + 1)
                    emit_wo(0, pts=(u - 12,), ob_eng="vector")
                else:
                    drain_o1(min(len(o1_ready) - 2, o1i + 2))

            # Phase D: tail. Remaining O(hp1,qf1) -> norm(1,qf1) ->
            # Wo(qf0) second half + Wo(qf1), fp16 copies + out DMA.
            drain_o1(len(o1_ready) - 1)
            emit_wo(0, pts=(2,), ob_eng="vector")
            drain_o1(len(o1_ready))
            emit_norm(1, 1)
            emit_wo(0, pts=(3,))
            emit_wo(1)

    nc.compile()
    return nc


# --------------------------------------------------------------------------
# public entry points
# --------------------------------------------------------------------------

def _prepare(inputs):
    q = np.ascontiguousarray(inputs["q_feat"], np.float32).reshape(B, C, N)
    kv = np.ascontiguousarray(inputs["kv_feat"], np.float32).reshape(B, C, N)
    offsets = np.asarray(inputs["offsets"], np.float32)
    sal = np.asarray(inputs["saliency_map"], np.float32).reshape(B, N)
    gamma = float(np.asarray(inputs["gamma_val"]))

    GTs = [_gather_T(offsets[b], 1.0 + gamma * sal[b]) for b in range(B)]

    # union band-sparsity pattern of the gather matmul across batches, so the
    # SPMD program is identical on every core
    chunks = []
    for m in range(NKT):
        for k in range(NKT):
            if any(GTs[b][k * P:(k + 1) * P, m * P:(m + 1) * P].any()
                   for b in range(B)):
                chunks.append((m, k))

    Wq = np.asarray(inputs["Wq"], np.float32) * (HD ** -0.5)
    Wk = np.asarray(inputs["Wk"], np.float32)
    Wv = np.asarray(inputs["Wv"], np.float32)
    Wo = np.asarray(inputs["Wo"], np.float32)

    def ptile(a):
        # [T*P, X] -> [P, T*X]: partition-tile-major columns
        t = a.shape[0] // P
        return np.ascontiguousarray(
            a.reshape(t, P, a.shape[1]).transpose(1, 0, 2).reshape(P, -1)
        ).astype(NP_BF16)

    in_maps = []
    for core in range(NCORES):
        b, g = core // G, core % G
        rows = slice(g * CG, (g + 1) * CG)
        gt_stack = np.stack([GTs[b][k * P:(k + 1) * P, m * P:(m + 1) * P]
                             for (m, k) in chunks])     # [nch, P, P]
        in_maps.append({
            "xq": ptile(q[b]),
            "xkv": ptile(kv[b]),
            "wqT": ptile(Wq[rows].T),
            "wkT": ptile(Wk[rows].T),
            "wvT": ptile(Wv[rows].T),
            "woT": ptile(Wo[:, rows].T),
            "gt": np.ascontiguousarray(
                gt_stack.transpose(1, 0, 2).reshape(P, -1)).astype(NP_BF16),
        })

    def assemble(results):
        out = np.empty((B, C, N), np.float32)
        for b in range(B):
            out[b] = (results[G * b]["outT"].astype(np.float32)
                      + results[G * b + 1]["outT"].astype(np.float32))
        return out.reshape(B, C, HH, WW)

    nc = _build_program(chunks)
    return nc, in_maps, assemble


def _needs_fallback(inputs):
    try:
        if tuple(np.shape(inputs["q_feat"])) != (B, C, HH, WW):
            return True
        for bias in ("bq", "bk", "bv", "bo"):
            if np.any(np.asarray(inputs[bias], np.float32) != 0.0):
                return True
    except Exception:
        return True
    return False


def kernel(**inputs) -> np.ndarray:
    if _needs_fallback(inputs):
        return _reference_numpy(**{k: np.asarray(v, np.float32)
                                   for k, v in inputs.items()})
    nc, in_maps, assemble = _prepare(inputs)
    res = run_bass_kernel_spmd(nc, in_maps, core_ids=list(range(NCORES)))
    return assemble(res.results)


def kernel_traced(trace_cores=(0,), **inputs):
    """Like kernel() but returns (output, exec_time_ns, trace_path)."""
    nc, in_maps, assemble = _prepare(inputs)
    res = run_bass_kernel_spmd(nc, in_maps, core_ids=list(range(NCORES)),
                               trace=True, trace_cores=list(trace_cores))
    trace_path = None
    if res.instructions_and_trace is not None:
        trace_path = res.instructions_and_trace[1]
    return assemble(res.results), res.exec_time_ns, trace_path
